# revision 18
# baseline (speedup 1.0000x reference)
"""Chamfer distance kernel for Trainium2 (8 NeuronCores via Bass/Tile).

Problem: B=4 batches of two 8192-point 3-D clouds (gt = coords+registration_gt,
pred = coords+registration_pred). Output scalar:
    mean_b(sum_n min_m D[n,m]) + mean_b(sum_m min_n D[n,m])
with D the squared-distance matrix of each batch.

Sharding: 8 cores = 4 batches x 2 directions (a direction's column-min is the
row-min of the transposed matrix, so every core solves the same row-min
problem on its own query/candidate pair).

v2 — spatial pruning. The brute-force 8192x8192 row-min is PSUM-evacuation
bound (~390us: DVE+ScalarE must stream all 64M distances at ~2 elem/lane/cyc).
Instead the host prunes candidates with a rigorous bound:
  1. Sort queries in Morton order; strips of 128 consecutive queries.
  2. Per query, a witness upper bound on its NN distance: min over a random
     256-candidate sample and a +/-32 window in candidate Morton order.
  3. Per strip, mark grid cells whose min distance to a query's cell is
     within that query's witness radius; the strip's candidate set = all
     candidates in marked cells. This is a guaranteed superset of every
     query's true NN (witness >= NN dist, cell bound <= true dist).
  4. Chunk each strip's set into slots of <=1024 candidates (rare heavy
     strips get several slots; host re-merges with min).
Device: per slot, 2 bf16 matmuls (K=12 hi/lo split features as before)
produce P'[q,c] = |C_c|^2 - 2 Q_q.C_c in a [128,1024] fp32 PSUM tile;
ScalarE stages the second half to SBUF; one custom-DVE MIN2 op computes
min(half0, half1) with a chained free-axis min-reduce into mins[:, slot].
~66 slots/core vs 512 full units -> ~8x less evacuation work.
"""

import numpy as np

B, C, N = 4, 3, 8192
PART = 128            # queries per slot (PSUM partition dim)
NC = 512              # candidates per slot (one PSUM bank fp32, one matmul)
KF = 12               # bf16 hi/lo split contraction depth

GRID_W = 0.2          # pruning grid cell width
WIT_SAMPLE = 256      # random-candidate witness sample size
WIT_WIN = 32          # morton-window witness half width
SENTINEL = 1.0e30     # |C|^2 feature value for padding candidates

_CACHE = {}


def _register_min2():
    """Register the custom DVE op MIN2_REDUCE_ANT at runtime:
    out = min(in0, in1); accum_out = min(s0, min_k out[k])."""
    import concourse.dve_ops as dve_ops
    from concourse.dve_spec import C0, Spec, Src0, Src1, _has_src1, lower, minn
    from concourse.dve_uop import DveOpSpec

    name = "MIN2_REDUCE_ANT"
    for op in dve_ops.OPS:
        if op.name == name:
            return op

    def _ref(in0, in1, s0, s1, imm2):
        b = np.minimum(in0.astype(np.float32), in1.astype(np.float32))
        m = b.reshape(b.shape[0], -1).min(axis=-1, keepdims=True)
        return b, np.minimum(s0, m)

    spec = Spec(body=minn(Src0, Src1), accum=minn, accum_init=C0, reference=_ref)
    row = max(dve_ops._SUB_OPCODE_FOR_NAME.values()) + 1
    assert row < 0x20
    dve_ops._SUB_OPCODE_FOR_NAME[name] = row
    shas = {}
    for ver in ("v3", "v4"):
        try:
            s = DveOpSpec(name=name, opcode=row, uops=lower(spec, ver=ver),
                          rd1_en=_has_src1(spec))
            shas[ver] = s.sha(ver)
        except Exception:
            pass
    op = dve_ops.DveOp(name, spec, subdim=False, uops_sha=shas)
    dve_ops.OPS.append(op)
    dve_ops.CUSTOM_DVE_SPECS[name] = spec
    return op


def _build_nc(n_slots):
    import concourse.bass as bass
    import concourse.tile as tile
    from concourse import bacc, mybir

    f32 = mybir.dt.float32
    bf16 = mybir.dt.bfloat16
    MIN2 = _register_min2()
    nc = bacc.Bacc("TRN2", target_bir_lowering=False, debug=False)

    qf = nc.declare_dram_parameter("qf", [KF, n_slots * PART], bf16, isOutput=False)
    cf = nc.declare_dram_parameter("cf", [KF, n_slots * NC], bf16, isOutput=False)
    mins = nc.declare_dram_parameter("mins", [PART, n_slots], f32, isOutput=True)

    with tile.TileContext(nc) as tc:
        with (
            tc.tile_pool(name="qin", bufs=1) as q_pool,
            tc.tile_pool(name="cin", bufs=6) as c_pool,
            tc.tile_pool(name="psum", bufs=4, space="PSUM") as psum_pool,
            tc.tile_pool(name="stage", bufs=6) as stage_pool,
            tc.tile_pool(name="scratch", bufs=6) as scratch_pool,
            tc.tile_pool(name="outbuf", bufs=1) as out_pool,
        ):
            # Query features replicated at row-group offsets 0 and 32 so the
            # two matmuls of a slot stream from independent PE row groups.
            # DMA issue (DIRECT2D descriptor gen) costs ~750ns serialized per
            # queue, so the loads are spread across 4 engine queues and the
            # first candidate batches are issued before the bulk qf load.
            qrep = q_pool.tile([128, n_slots * PART], bf16)
            minsbuf = out_pool.tile([PART, n_slots], f32)

            DB = 4
            assert n_slots % DB == 0
            n_batches = n_slots // DB
            creps = {}

            def load_crep(batch, q0=None, q1=None):
                crep_b = c_pool.tile([128, DB * NC], bf16)
                src = cf[:, batch * DB * NC : (batch + 1) * DB * NC]
                (q0 or nc.sync).dma_start(out=crep_b[0:KF, :], in_=src)
                (q1 or nc.scalar).dma_start(out=crep_b[32 : 32 + KF, :], in_=src)
                creps[batch] = crep_b

            # batch 0 + slot-0 query weights entirely on the sync queue: the
            # scalar queue is blocked by ACT_TABLE_LOAD early on.
            load_crep(0, q0=nc.sync, q1=nc.sync)
            QSPLIT = 8 * PART
            nc.sync.dma_start(out=qrep[0:KF, 0:QSPLIT], in_=qf[:, 0:QSPLIT])
            nc.sync.dma_start(out=qrep[32 : 32 + KF, 0:QSPLIT], in_=qf[:, 0:QSPLIT])
            load_crep(1)
            for rg, q in ((0, nc.sync), (1, nc.scalar)):
                q.dma_start(
                    out=qrep[32 * rg : 32 * rg + KF, QSPLIT:],
                    in_=qf[:, QSPLIT:],
                )
            for b in range(2, min(5, n_batches)):
                load_crep(b)

            # Slot pairs share a [128, 2*NC] PSUM tile (2 banks). Each slot is
            # one 512-col matmul; the two staged quarters sit contiguously in
            # the middle (cols NC/2 .. 3NC/2), so ONE ScalarE copy per pair
            # stages both, then two FD=NC/2 MIN2s reduce the pair.
            H = NC // 2
            pdp = None
            for s in range(n_slots):
                if s % DB == 0 and 5 <= s // DB + 5 < n_batches:
                    load_crep(s // DB + 5)
                crep_b = creps[s // DB]
                c0 = (s % DB) * NC
                rg = s % 2
                if rg == 0:
                    pdp = psum_pool.tile([128, 2 * NC], f32, tag="pd")
                nc.tensor.matmul(
                    pdp[:, rg * NC : (rg + 1) * NC],
                    qrep[32 * rg : 32 * rg + KF, s * PART : (s + 1) * PART],
                    crep_b[32 * rg : 32 * rg + KF, c0 : c0 + NC],
                    start=True,
                    stop=True,
                    tile_position=(32 * rg, 0),
                )
                if rg == 1:
                    st = stage_pool.tile([128, NC], f32)
                    nc.scalar.copy(st[:], pdp[:, H : H + NC])
                    for k in (0, 1):
                        sc = scratch_pool.tile([128, H], f32)
                        nc.vector._custom_dve(
                            MIN2,
                            out=sc[:],
                            in0=pdp[:, 0:H] if k == 0 else pdp[:, NC + H : 2 * NC],
                            in1=st[:, 0:H] if k == 0 else st[:, H:NC],
                            s0=3.0e38,
                            s1=0.0,
                            accum_out=minsbuf[:, s - 1 + k : s + k],
                        )

            nc.sync.dma_start(out=mins[:, :], in_=minsbuf[:])

    nc.finalize()
    return nc


# ---------------- host-side pruning ----------------


def _morton3(c, bits=7):
    out = np.zeros(len(c), dtype=np.int64)
    for b in range(bits):
        for j in range(3):
            out |= ((c[:, j] >> b) & 1) << (3 * b + (2 - j))
    return out


def _prune_core(Q, Cc, seed=0):
    """Q, Cc: [3, N] float32 query/candidate clouds.
    Returns (qperm, slots) where slots is a list of (strip_idx, cand_idx array
    of length<=NC). Candidate sets are guaranteed supersets of each strip
    query's true nearest neighbor."""
    w = GRID_W
    Qt = Q.T.astype(np.float64)
    Ct = Cc.T.astype(np.float64)
    n = len(Qt)
    lo = np.minimum(Qt.min(0), Ct.min(0)) - 1e-6
    cq = np.floor((Qt - lo) / w).astype(np.int64)
    cc = np.floor((Ct - lo) / w).astype(np.int64)
    G = int(max(cq.max(), cc.max())) + 2
    moq = _morton3(cq)
    moc = _morton3(cc)
    qperm = np.argsort(moq, kind="stable")
    Qs = Qt[qperm]
    cperm = np.argsort(moc, kind="stable")
    Cs = Ct[cperm]
    moc_s = moc[cperm]

    # witness upper bound on NN distance: random sample + morton window
    rng = np.random.default_rng(seed)
    samp = Ct[rng.choice(n, WIT_SAMPLE, replace=False)]
    wit2 = ((Qs[:, None, :] - samp[None, :, :]) ** 2).sum(-1).min(1)
    pos = np.searchsorted(moc_s, moq[qperm])
    idx = np.clip(pos[:, None] + np.arange(-WIT_WIN, WIT_WIN)[None, :], 0, n - 1)
    dw2 = ((Qs[:, None, :] - Cs[idx]) ** 2).sum(-1).min(1)
    wit = np.sqrt(np.minimum(wit2, dw2)) * (1 + 1e-6) + 1e-9

    cqs = cq[qperm]
    cc_flat = cc[:, 0] * G * G + cc[:, 1] * G + cc[:, 2]
    GG = G * G * G

    # global offset table up to the largest witness radius, sorted by the
    # cell-to-cell lower-bound distance need(o) = sum_i max(|o_i|-1,0)^2 w^2
    kglob = int(np.ceil(wit.max() / w)) + 1
    r = np.arange(-kglob, kglob + 1)
    ox, oy, oz = np.meshgrid(r, r, r, indexing="ij")
    off = np.stack([ox.ravel(), oy.ravel(), oz.ravel()], 1)
    need = (np.maximum(np.abs(off) - 1, 0) ** 2).sum(1).astype(np.float64) * w * w
    osort = np.argsort(need, kind="stable")
    off = off[osort]
    need = need[osort]

    slots = []
    n_strips = n // PART
    for s in range(n_strips):
        q0 = s * PART
        wv2 = wit[q0 : q0 + PART] ** 2
        order = np.argsort(-wv2, kind="stable")
        wv2s = wv2[order]
        base = cqs[q0 : q0 + PART][order]
        omax = int(np.searchsorted(need, wv2s[0], side="right"))
        mark = np.zeros(GG, dtype=bool)
        for o in range(omax):
            cnt = int(np.searchsorted(-wv2s, -need[o], side="right"))
            if cnt == 0:
                break
            cx = base[:cnt, 0] + off[o, 0]
            cy = base[:cnt, 1] + off[o, 1]
            cz = base[:cnt, 2] + off[o, 2]
            ok = ((cx >= 0) & (cx < G) & (cy >= 0) & (cy < G)
                  & (cz >= 0) & (cz < G))
            if ok.any():
                mark[cx[ok] * G * G + cy[ok] * G + cz[ok]] = True
        cand = np.nonzero(mark[cc_flat])[0]
        assert len(cand) > 0
        for c0 in range(0, len(cand), NC):
            slots.append((s, cand[c0 : c0 + NC]))
    return qperm, slots


def _features_q(Q):
    """[12, n] query-side rows: P' = qfeat.T @ cfeat."""
    import ml_dtypes

    bf16 = ml_dtypes.bfloat16
    Q = Q.astype(np.float32)
    qh = Q.astype(bf16).astype(np.float32)
    ql = (Q - qh).astype(bf16).astype(np.float32)
    ones = np.ones((3, Q.shape[1]), np.float32)
    qf = np.concatenate([-2 * qh, -2 * qh, -2 * ql, ones], axis=0)
    return np.ascontiguousarray(qf.astype(bf16))


def _features_c(Cc):
    """[12, n] candidate-side rows."""
    import ml_dtypes

    bf16 = ml_dtypes.bfloat16
    Cc = Cc.astype(np.float32)
    ch = Cc.astype(bf16).astype(np.float32)
    cl = (Cc - ch).astype(bf16).astype(np.float32)
    sq2 = (Cc.astype(np.float64) ** 2).sum(axis=0).astype(np.float32)[None, :]
    s1 = sq2.astype(bf16).astype(np.float32)
    s2 = (sq2 - s1).astype(bf16).astype(np.float32)
    s3 = (sq2 - s1 - s2).astype(bf16).astype(np.float32)
    cfe = np.concatenate([ch, cl, ch, s1, s2, s3], axis=0)
    return np.ascontiguousarray(cfe.astype(bf16))


def _host_inputs(registration_pred, registration_gt, coords):
    """Per-core input maps + combine metadata. Core 2*b+d: batch b, direction
    d (d=0: queries=gt cloud, candidates=pred cloud; d=1: swapped)."""
    import ml_dtypes

    bf16 = ml_dtypes.bfloat16
    pc_gt = (coords + registration_gt).astype(np.float32)
    pc_pr = (coords + registration_pred).astype(np.float32)

    cores = []
    for b in range(B):
        for d in range(2):
            Q = pc_gt[b] if d == 0 else pc_pr[b]
            Cc = pc_pr[b] if d == 0 else pc_gt[b]
            qperm, slots = _prune_core(Q, Cc, seed=17 * b + d)
            cores.append((Q, Cc, qperm, slots))

    n_slots = max(len(sl) for (_, _, _, sl) in cores)
    n_slots = -(-n_slots // 4) * 4  # round up to multiple of 4

    in_maps = []
    metas = []
    for Q, Cc, qperm, slots in cores:
        qf_all = _features_q(Q[:, qperm])          # [12, N] in strip order
        cf_all = _features_c(Cc)                   # [12, N] original order
        qf = np.zeros((KF, n_slots * PART), dtype=bf16)
        cf = np.zeros((KF, n_slots * NC), dtype=bf16)
        cf[9:12, :] = np.float32(SENTINEL)         # sentinel pad: P' = 3e30
        for j, (s, cand) in enumerate(slots):
            qf[:, j * PART : (j + 1) * PART] = qf_all[:, s * PART : (s + 1) * PART]
            cf[:, j * NC : j * NC + len(cand)] = cf_all[:, cand]
        in_maps.append({"qf": qf, "cf": cf})
        qsq = float((Q.astype(np.float64) ** 2).sum())
        metas.append((qsq, [s for (s, _) in slots]))
    return in_maps, metas


def _combine(results, metas):
    per_core = []
    for i in range(2 * B):
        qsq, slot_strips = metas[i]
        m = results[i]["mins"].astype(np.float64)  # [128, n_slots]
        n_strips = N // PART
        best = np.full((PART, n_strips), np.inf)
        for j, s in enumerate(slot_strips):
            best[:, s] = np.minimum(best[:, s], m[:, j])
        per_core.append(best.sum() + qsq)
    d1 = sum(per_core[2 * b] for b in range(B)) / B
    d2 = sum(per_core[2 * b + 1] for b in range(B)) / B
    return np.array(d1 + d2, dtype=np.float32)


def kernel(registration_pred, registration_gt, coords):
    from concourse.bass_utils import run_bass_kernel_spmd

    registration_pred = np.asarray(registration_pred, np.float32)
    registration_gt = np.asarray(registration_gt, np.float32)
    coords = np.asarray(coords, np.float32)

    in_maps, metas = _host_inputs(registration_pred, registration_gt, coords)
    n_slots = in_maps[0]["qf"].shape[1] // PART
    key = ("nc", n_slots)
    if key not in _CACHE:
        _CACHE[key] = _build_nc(n_slots)
    nc = _CACHE[key]
    _CACHE["nc"] = nc
    _CACHE["n_slots"] = n_slots

    res = run_bass_kernel_spmd(nc, in_maps, core_ids=list(range(2 * B)))
    return _combine(res.results, metas)
